# revision 1
# baseline (speedup 1.0000x reference)
"""CFConv (SchNet continuous-filter conv) Trainium2 Bass kernel, 8-core SPMD.

Strategy:
  - Host: bucket edges by destination node range (ind_i // 6250 -> core),
    within core group by (128-node dest window, src-half) so that
    segment_sum becomes one-hot matmuls accumulated in PSUM per window,
    and h-row gathers use int16 indices (j < 25000 per half).
  - Device (per core): h = x @ Win -> HBM; per 512-edge supertile:
    load f^T block, dma_gather h rows, filter MLP (softplus on ACT),
    cosine cutoff folded into the one-hot, scatter via PE matmul,
    per-window output MLP.
No cross-core collectives: each core owns 6250 output rows.
"""

import math
import os
import sys

import numpy as np

sys.path.insert(0, "/opt/trn_rl_repo")

N_ATOMS = 50000
N_EDGES = 1600000
DIM = 128
NF = 128
NG = 50
CUTOFF = 10.0
LOG2 = float(np.log(2.0))
NCORES = 8
NPC = N_ATOMS // NCORES  # 6250 nodes per core
WIN = 128  # dest-window size (scatter matmul M dim)
NWIN = (NPC + WIN - 1) // WIN  # 49
JHALF = 25000  # gather index half size (int16 limit)
SUPER = 512  # edges per supertile
NB = SUPER // 128  # 4 blocks per supertile
NPAD = 50176  # padded atom count for h phase (392*128)

F32 = None  # set after mybir import
_cache = {}


def _prep(inputs):
    """Host-side bucketing/padding. Returns per-core arrays + constants."""
    x = np.ascontiguousarray(np.asarray(inputs["x"], dtype=np.float32))
    r = np.asarray(inputs["r_ij"], dtype=np.float32)
    f = np.asarray(inputs["f_ij"], dtype=np.float32)
    ii = np.asarray(inputs["ind_i"]).astype(np.int64)
    jj = np.asarray(inputs["ind_j"]).astype(np.int64)

    core = ii // NPC
    wloc = (ii - core * NPC) // WIN  # 0..48
    lloc = (ii - core * NPC - wloc * WIN).astype(np.float32)  # 0..127
    half = (jj >= JHALF).astype(np.int64)
    wg = wloc * 2 + half  # group id within core, 0..97
    NG_GROUPS = NWIN * 2

    gkey = core * NG_GROUPS + wg
    order = np.lexsort((jj, gkey))  # sort by (core, window, half), then j
    counts = np.bincount(gkey, minlength=NCORES * NG_GROUPS).reshape(
        NCORES, NG_GROUPS
    )
    gmax = counts.max(axis=0)
    gpad = np.maximum(SUPER, ((gmax + SUPER - 1) // SUPER) * SUPER)  # [98]
    offs = np.concatenate([[0], np.cumsum(gpad)])
    E_pad = int(offs[-1])
    T_cols = E_pad // 128

    # destination slot for each edge (in sorted order)
    sorted_gkey = gkey[order]
    # rank within each (core, group)
    first_idx = np.searchsorted(sorted_gkey, np.arange(NCORES * NG_GROUPS))
    rank = np.arange(N_EDGES) - first_idx[sorted_gkey]
    slot = offs[sorted_gkey % NG_GROUPS] + rank  # position within core's E_pad

    per_core = []
    for c in range(NCORES):
        sel = order[core[order] == c]
        sl = slot[core[order] == c]
        # padded slot arrays
        f_pad = np.zeros((E_pad, NG), dtype=np.float32)
        r_pad = np.full(E_pad, 15.0, dtype=np.float32)  # killed by r<10 mask
        l_pad = np.zeros(E_pad, dtype=np.float32)
        j_pad = np.zeros(E_pad, dtype=np.int16)
        f_pad[sl] = f[sel]
        r_pad[sl] = r[sel]
        l_pad[sl] = lloc[sel]
        j_pad[sl] = (jj[sel] - half[sel] * JHALF).astype(np.int16)
        per_core.append(
            dict(
                fT=np.ascontiguousarray(f_pad.T),  # [50, E_pad]
                rA=np.ascontiguousarray(r_pad.reshape(T_cols, 128).T),  # [128,T]
                lA=np.ascontiguousarray(l_pad.reshape(T_cols, 128).T),  # [128,T]
                jx=np.ascontiguousarray(
                    np.tile(j_pad.reshape(-1, 16).T, (8, 1))
                ),  # [128, E_pad//16]
            )
        )

    xT = np.zeros((DIM, NPAD), dtype=np.float32)
    xT[:, :N_ATOMS] = x.T
    consts = dict(
        xT=np.ascontiguousarray(xT),
        Wf1=np.asarray(inputs["Wf1"], dtype=np.float32),
        Wf2=np.asarray(inputs["Wf2"], dtype=np.float32),
        Win=np.ascontiguousarray(np.asarray(inputs["Win"], dtype=np.float32)),
        Wout=np.ascontiguousarray(np.asarray(inputs["Wout"], dtype=np.float32)),
        b1=np.asarray(inputs["bf1"], dtype=np.float32).reshape(NF, 1),
        b2x4=np.ascontiguousarray(
            np.tile(
                (
                    np.asarray(inputs["bf2"], dtype=np.float32)
                    - LOG2 * np.asarray(inputs["Wf2"], dtype=np.float32).sum(0)
                ).reshape(1, NF),
                (1, NB),
            )
        ),  # [1, 512]
        bout=np.asarray(inputs["bout"], dtype=np.float32).reshape(1, NF),
        ones=np.ones((1, 128), dtype=np.float32),
        iota=np.ascontiguousarray(
            np.broadcast_to(np.arange(128, dtype=np.float32), (128, 128)).copy()
        ),
        ident=np.eye(128, dtype=np.float32),
    )
    return per_core, consts, gpad, E_pad, T_cols


def _build(gpad, E_pad, T_cols, stat_bf16=True, bout_nonzero=False,
           bf2_like_nonzero=True, dbg_no_gather=False):
    """Build the SPMD bass program (same for all cores)."""
    from contextlib import ExitStack

    import concourse.bacc as bacc
    import concourse.bass as bass
    import concourse.mybir as mybir
    import concourse.tile as tile

    dt = mybir.dt
    AF = mybir.ActivationFunctionType
    OP = mybir.AluOpType
    DT_STAT = dt.bfloat16 if stat_bf16 else dt.float32

    nc = bacc.Bacc()

    # ---- I/O ----
    fT_d = nc.declare_dram_parameter("fT", [NG, E_pad], DT_STAT, isOutput=False)
    rA_d = nc.declare_dram_parameter("rA", [128, T_cols], dt.float32, isOutput=False)
    lA_d = nc.declare_dram_parameter("lA", [128, T_cols], dt.float32, isOutput=False)
    jx_d = nc.declare_dram_parameter(
        "jx", [128, E_pad // 16], dt.int16, isOutput=False
    )
    xT_d = nc.declare_dram_parameter("xT", [DIM, NPAD], dt.float32, isOutput=False)
    Wf1_d = nc.declare_dram_parameter("Wf1", [NG, NF], DT_STAT, isOutput=False)
    Wf2_d = nc.declare_dram_parameter("Wf2", [NF, NF], DT_STAT, isOutput=False)
    Win_d = nc.declare_dram_parameter("Win", [DIM, NF], dt.float32, isOutput=False)
    Wout_d = nc.declare_dram_parameter("Wout", [NF, NF], dt.float32, isOutput=False)
    b1_d = nc.declare_dram_parameter("b1", [NF, 1], dt.float32, isOutput=False)
    b2_d = nc.declare_dram_parameter("b2x4", [1, NB * NF], dt.float32, isOutput=False)
    bout_d = nc.declare_dram_parameter("bout", [1, NF], dt.float32, isOutput=False)
    ones_d = nc.declare_dram_parameter("ones", [1, 128], dt.float32, isOutput=False)
    iota_d = nc.declare_dram_parameter("iota", [128, 128], dt.float32, isOutput=False)
    ident_d = nc.declare_dram_parameter(
        "ident", [128, 128], dt.float32, isOutput=False
    )
    out_d = nc.declare_dram_parameter("out", [NPC, NF], dt.float32, isOutput=True)

    h_d = nc.dram_tensor("h", [NPAD, NF], dt.float32)

    offs = np.concatenate([[0], np.cumsum(gpad)]).astype(int)

    with tile.TileContext(nc) as tc, ExitStack() as ctx:
        cpool = ctx.enter_context(tc.tile_pool(name="consts", bufs=1))
        meta = ctx.enter_context(tc.tile_pool(name="meta", bufs=1))
        xpool = ctx.enter_context(tc.tile_pool(name="xload", bufs=3))
        hspool = ctx.enter_context(tc.tile_pool(name="hstage", bufs=3))
        ftpool = ctx.enter_context(tc.tile_pool(name="ft", bufs=3))
        apool = ctx.enter_context(tc.tile_pool(name="a1", bufs=3))
        epool = ctx.enter_context(tc.tile_pool(name="e1", bufs=3))
        tpool = ctx.enter_context(tc.tile_pool(name="tt", bufs=3))
        mpool = ctx.enter_context(tc.tile_pool(name="m0", bufs=3))
        ohpool = ctx.enter_context(tc.tile_pool(name="oh", bufs=6))
        hgpool = ctx.enter_context(tc.tile_pool(name="hg", bufs=2))
        opool = ctx.enter_context(tc.tile_pool(name="outs", bufs=3))
        pz = ctx.enter_context(
            tc.tile_pool(name="pz", bufs=2, space=bass.MemorySpace.PSUM)
        )
        pz2 = ctx.enter_context(
            tc.tile_pool(name="pz2", bufs=2, space=bass.MemorySpace.PSUM)
        )
        pagg = ctx.enter_context(
            tc.tile_pool(name="pagg", bufs=2, space=bass.MemorySpace.PSUM)
        )
        pout = ctx.enter_context(
            tc.tile_pool(name="pout", bufs=1, space=bass.MemorySpace.PSUM)
        )

        # ---- constants into SBUF ----
        Wf1 = cpool.tile([NG, NF], DT_STAT)
        nc.sync.dma_start(Wf1[:], Wf1_d[:])
        Wf2 = cpool.tile([NF, NF], DT_STAT)
        nc.sync.dma_start(Wf2[:], Wf2_d[:])
        Win = cpool.tile([DIM, NF], dt.float32)
        nc.sync.dma_start(Win[:], Win_d[:])
        Wout = cpool.tile([NF, NF], dt.float32)
        nc.sync.dma_start(Wout[:], Wout_d[:])
        b1 = cpool.tile([NF, 1], dt.float32)
        nc.sync.dma_start(b1[:], b1_d[:])
        b2x4 = cpool.tile([1, NB * NF], dt.float32)
        nc.sync.dma_start(b2x4[:], b2_d[:])
        bout = cpool.tile([1, NF], dt.float32)
        nc.sync.dma_start(bout[:], bout_d[:])
        ones = cpool.tile([1, 128], dt.float32)
        nc.sync.dma_start(ones[:], ones_d[:])
        iota = cpool.tile([128, 128], dt.float32)
        nc.sync.dma_start(iota[:], iota_d[:])
        ident = cpool.tile([128, 128], dt.float32)
        nc.sync.dma_start(ident[:], ident_d[:])

        # ---- per-edge metadata: l, C ----
        lA = meta.tile([128, T_cols], dt.float32)
        nc.sync.dma_start(lA[:], lA_d[:])
        rA = meta.tile([128, T_cols], dt.float32)
        nc.sync.dma_start(rA[:], rA_d[:])
        jx = meta.tile([128, E_pad // 16], dt.int16)
        nc.sync.dma_start(jx[:], jx_d[:])

        CA = meta.tile([128, T_cols], dt.float32)
        # cos(pi*r/10) = sin(pi/2 - pi*r/10), argument in [-pi, pi] for
        # r in [0, 15] (ACT Sin spline domain); C = (0.5*C0+0.5) * (r < 10)
        rS = meta.tile([128, T_cols], dt.float32)
        nc.vector.tensor_scalar(
            rS[:], rA[:], float(-np.pi / CUTOFF), float(np.pi / 2), OP.mult, OP.add
        )
        nc.scalar.activation(CA[:], rS[:], AF.Sin)
        nc.vector.tensor_scalar(CA[:], CA[:], 0.5, 0.5, OP.mult, OP.add)
        msk = meta.tile([128, T_cols], dt.float32)
        nc.vector.tensor_scalar(msk[:], rA[:], float(CUTOFF), None, OP.is_lt)
        nc.vector.tensor_tensor(CA[:], CA[:], msk[:], OP.mult)

        # ---- phase 1: h = x @ Win  ([NPAD, 128] fp32 in DRAM) ----
        for nb in range(NPAD // SUPER):  # 98 groups of 4 node-blocks
            xa = xpool.tile([DIM, SUPER], dt.float32)
            nc.sync.dma_start(xa[:], xT_d[:, nb * SUPER : (nb + 1) * SUPER])
            hp = pz.tile([128, SUPER], dt.float32, tag="z1")
            for b in range(NB):
                nc.tensor.matmul(
                    hp[:, b * 128 : (b + 1) * 128],
                    xa[:, b * 128 : (b + 1) * 128],
                    Win[:],
                    start=True,
                    stop=True,
                )
            hs = hspool.tile([128, NB, 128], dt.float32)
            nc.scalar.copy(hs[:], hp[:].rearrange("p (b f) -> p b f", b=NB))
            nc.sync.dma_start(
                h_d[nb * SUPER : (nb + 1) * SUPER, :].rearrange(
                    "(b p) f -> p b f", p=128
                ),
                hs[:],
            )

        tc.strict_bb_all_engine_barrier()

        # ---- phase 2: edge loop ----
        h_lo = h_d[0:JHALF, :]
        h_hi = h_d[JHALF : 2 * JHALF, :]
        for w in range(NWIN):
            agg = pagg.tile([128, NF], dt.float32)
            n_tiles_w = (gpad[2 * w] + gpad[2 * w + 1]) // 128
            tile_i = 0  # running tile index within window
            for hh in range(2):
                g = 2 * w + hh
                gsz = int(gpad[g])
                goff = int(offs[g])
                hg = hgpool.tile([128, gsz // 128, 128], dt.float32, tag="hg")
                if dbg_no_gather:
                    nc.gpsimd.memset(hg[:], 1.0)
                else:
                    nc.gpsimd.dma_gather(
                        hg[:],
                        h_lo if hh == 0 else h_hi,
                        jx[:, goff // 16 : (goff + gsz) // 16],
                        gsz,
                        gsz,
                        NF,
                        single_packet=False,
                    )
                for s in range(gsz // SUPER):
                    e0 = goff + s * SUPER
                    ft = ftpool.tile([NG, SUPER], DT_STAT)
                    nc.sync.dma_start(ft[:], fT_d[:, e0 : e0 + SUPER])
                    z1 = pz.tile([128, SUPER], dt.float32)
                    nc.tensor.matmul(z1[:], Wf1[:], ft[:], start=True, stop=True)
                    # softplus = ln(1 + exp(.)) — Softplus has no ACT table
                    # in this toolchain; exp+ln live in one table set.
                    e1 = epool.tile([128, SUPER], dt.float32)
                    nc.scalar.activation(e1[:], z1[:], AF.Exp, bias=b1[:, 0:1])
                    a1 = apool.tile([128, NB, 128], DT_STAT)
                    nc.scalar.activation(
                        a1[:],
                        e1[:].rearrange("p (b f) -> p b f", b=NB),
                        AF.Ln,
                        bias=1.0,
                    )
                    z2 = pz2.tile([128, NB, 128], dt.float32)
                    # start=True zeroes the whole 2KB bank region; the bias
                    # matmul fills all 512 cols, block matmuls accumulate.
                    nc.tensor.matmul(
                        z2[:], ones[:], b2x4[:], start=True, stop=False
                    )
                    for b in range(NB):
                        nc.tensor.matmul(
                            z2[:, b, :],
                            a1[:, b, :],
                            Wf2[:],
                            start=False,
                            stop=(b == NB - 1),
                        )
                    e2 = epool.tile([128, NB, 128], dt.float32, tag="e2")
                    nc.scalar.activation(e2[:], z2[:], AF.Exp)
                    tt = tpool.tile([128, NB, 128], dt.float32)
                    nc.scalar.activation(tt[:], e2[:], AF.Ln, bias=1.0)
                    # T' = T - log2 (outer ssp shift)
                    tq = tpool.tile([128, NB, 128], dt.float32, tag="tq")
                    nc.vector.tensor_scalar(tq[:], tt[:], LOG2, None, OP.subtract)
                    m0 = mpool.tile([128, NB, 128], DT_STAT)
                    nc.vector.tensor_tensor(
                        m0[:], tq[:], hg[:, s * NB : (s + 1) * NB, :], OP.mult
                    )
                    for b in range(NB):
                        tcol = e0 // 128 + b
                        oh = ohpool.tile([128, 128], DT_STAT)
                        nc.vector.tensor_scalar(
                            oh[:],
                            iota[:],
                            lA[:, tcol : tcol + 1],
                            CA[:, tcol : tcol + 1],
                            OP.is_equal,
                            OP.mult,
                        )
                        nc.tensor.matmul(
                            agg[:],
                            oh[:],
                            m0[:, b, :],
                            start=(tile_i == 0),
                            stop=(tile_i == n_tiles_w - 1),
                        )
                        tile_i += 1

            # ---- window output: out_w = ssp(agg @ Wout + bout) ----
            aggs = opool.tile([128, NF], dt.float32)
            nc.scalar.copy(aggs[:], agg[:])
            aggTp = pout.tile([128, 128], dt.float32)
            nc.tensor.transpose(aggTp[:], aggs[:], ident[:])
            aggT = opool.tile([128, 128], dt.float32)
            nc.vector.tensor_copy(aggT[:], aggTp[:])
            op = pout.tile([128, NF], dt.float32, tag="op")
            if bout_nonzero:
                nc.tensor.matmul(op[:], ones[:], bout[:], start=True, stop=False)
                nc.tensor.matmul(
                    op[:], aggT[:], Wout[:], start=False, stop=True
                )
            else:
                nc.tensor.matmul(op[:], aggT[:], Wout[:], start=True, stop=True)
            eo = opool.tile([128, NF], dt.float32, tag="eo")
            nc.scalar.activation(eo[:], op[:], AF.Exp)
            outs = opool.tile([128, NF], dt.float32)
            nc.scalar.activation(outs[:], eo[:], AF.Ln, bias=1.0)
            outs2 = opool.tile([128, NF], dt.float32, tag="outs2")
            nc.vector.tensor_scalar(outs2[:], outs[:], LOG2, None, OP.subtract)
            nrows = min(WIN, NPC - w * WIN)
            nc.sync.dma_start(
                out_d[w * WIN : w * WIN + nrows, :], outs2[:nrows, :]
            )

    if not nc.is_finalized():
        nc.finalize()
    return nc


def kernel(**inputs):
    from concourse.bass_utils import run_bass_kernel_spmd

    per_core, consts, gpad, E_pad, T_cols = _prep(inputs)

    stat_bf16 = os.environ.get("CFCONV_F32", "0") != "1"
    bout_nonzero = bool(np.any(consts["bout"]))
    np_stat = np.dtype("float32")
    if stat_bf16:
        import ml_dtypes

        np_stat = np.dtype(ml_dtypes.bfloat16)

    nc = _build(gpad, E_pad, T_cols, stat_bf16=stat_bf16,
                bout_nonzero=bout_nonzero)

    in_maps = []
    for c in range(NCORES):
        m = dict(per_core[c])
        m["fT"] = np.ascontiguousarray(m["fT"].astype(np_stat))
        m.update(consts)
        m["Wf1"] = consts["Wf1"].astype(np_stat)
        m["Wf2"] = consts["Wf2"].astype(np_stat)
        in_maps.append(m)

    trace = os.environ.get("CFCONV_TRACE", "0") == "1"
    res = run_bass_kernel_spmd(nc, in_maps, list(range(NCORES)), trace=trace)
    if trace and res.exec_time_ns is not None:
        print(f"HW exec time: {res.exec_time_ns} ns")
        kernel.last_exec_time_ns = res.exec_time_ns
    kernel.last_results = res
    out = np.concatenate(
        [np.asarray(res.results[c]["out"]) for c in range(NCORES)], axis=0
    )
    return out.astype(np.float32)

